# revision 17
# baseline (speedup 1.0000x reference)
"""ConvAttention Trainium2 kernel.

Full-input contract: kernel(**inputs) takes the complete unsharded inputs
(x: (8, 512, 32, 32), gamma: (1, 512, 1, 1), w_qkv: (1536, 512),
w_out: (512, 512)) and returns the full (8, 512, 32, 32) output.

Sharding: data-parallel over batch — core b computes batch element b
entirely on-chip. No collectives.

Per-core math (b fixed), everything kept in [channel(part), spatial(free)]
layout:
  xn = LayerNorm_c(x) * gamma      (stats via ones-matmul rows on PE,
                                    row stats broadcast via K=1 matmuls)
  q,k = W_qk @ xn ; vT = xn^T W_v^T (computed directly transposed)
  per head: simT = k^T q; expT = exp(simT * s)  (softmax w/o max: logits
            are O(5), fp32 exp is safe)
  out_aug = [v;1]-augmented matmul -> numerator rows + denominator row
  attn_out = numerator * broadcast(1/denominator)
  y = w_out @ attn_out + x

Matmuls run in float32r (~1.5e-4 rel err, 4x fp32 throughput).
HW constraints honored: matmul operand/output base partitions and all AP
partition offsets are 32-aligned; f32r matmul inputs are produced by
compute ops with declared f32r outputs.
"""

import numpy as np

C = 512
N = 1024
O3 = 1536
H = 8
DH = 64
EPS = 1e-5
SCALE = 64.0 ** -0.5
NCORES = 8

_CACHE = {}


def _build():
    import concourse.bacc as bacc
    import concourse.tile as tile
    from concourse import mybir
    from concourse.masks import make_identity

    f32 = mybir.dt.float32
    f32r = mybir.dt.float32r
    AF = mybir.ActivationFunctionType
    OP = mybir.AluOpType

    nc = bacc.Bacc("TRN2", target_bir_lowering=False, debug=False, num_devices=1)
    x_ap = nc.dram_tensor("x", [C, N], f32, kind="ExternalInput").ap()
    g_ap = nc.dram_tensor("gamma", [C], f32, kind="ExternalInput").ap()
    wqkv_ap = nc.dram_tensor("w_qkv", [O3, C], f32, kind="ExternalInput").ap()
    wout_ap = nc.dram_tensor("w_out", [C, C], f32, kind="ExternalInput").ap()
    y_ap = nc.dram_tensor("y", [C, N], f32, kind="ExternalOutput").ap()

    mm = nc.tensor.matmul

    with tile.TileContext(nc) as tc:
        with (
            tc.tile_pool(name="const", bufs=1) as const,
            tc.tile_pool(name="xin", bufs=1) as xin,
            tc.tile_pool(name="acts", bufs=1) as acts,
            tc.tile_pool(name="wTp", bufs=1) as wTp,
            tc.tile_pool(name="rows", bufs=1) as rows,
        ):
            ident = const.tile([128, 128], f32)
            make_identity(nc, ident)
            ones_f = const.tile([128, 1], f32)
            nc.vector.memset(ones_f, 1.0)
            ones_col = const.tile([128, 1], f32r)
            nc.scalar.copy(out=ones_col, in_=ones_f)
            # row operands for K=1 broadcast matmuls (base partition 0 only)
            onesr_f = const.tile([1, DH], f32)
            nc.vector.memset(onesr_f, 1.0)
            ones_row = const.tile([1, DH], f32r)
            nc.scalar.copy(out=ones_row, in_=onesr_f)
            gamma_f = const.tile([1, C], f32)
            nc.sync.dma_start(out=gamma_f, in_=g_ap[None, :])
            gamma_row = const.tile([1, C], f32r)
            nc.scalar.copy(out=gamma_row, in_=gamma_f)
            vones_f = const.tile([128, 8, H, 1], f32)
            nc.vector.memset(vones_f, 1.0)
            eps_col = const.tile([65, 1], f32)
            nc.vector.memset(eps_col, EPS)

            # ---- load x ----
            x_sb = xin.tile([128, 4, N], f32)
            for t in range(4):
                nc.sync.dma_start(
                    out=x_sb[:, t, :], in_=x_ap[t * 128 : (t + 1) * 128, :]
                )

            # ---- persistent activation tiles ----
            qk_sb = acts.tile([128, 8, N], f32r)  # q: 0..3, k: 4..7
            vT_sb = acts.tile([128, 8, H, DH + 1], f32r)  # [nt, h, dh|ones]
            att_sb = acts.tile([128, 4, N], f32r)
            nc.scalar.copy(out=vT_sb[:, :, :, DH : DH + 1], in_=vones_f)

            # weights, transposed ([contraction-part, out-free])
            wqkT = wTp.tile([128, 4, 1024], f32r)
            wvT = wTp.tile([128, 4, C], f32r)
            woT = wTp.tile([128, 4, C], f32r)

            # stat rows, packed at 32-aligned partitions of shared tiles
            stA = rows.tile([97, N], f32)  # mean@0, msq@32, var@64, sd@96
            stB = rows.tile([97, N], f32)  # a@0, b@32, recip-tmp@64
            st_ra = rows.tile([1, N], f32r)  # a (rounded), base 0
            st_rb = rows.tile([1, N], f32r)  # b (rounded), base 0
            mean_r, msq_r, var_r, sd_r = (
                stA[0:1, :],
                stA[32:33, :],
                stA[64:65, :],
                stA[96:97, :],
            )
            a_r, b_r, rtmp = stB[0:1, :], stB[32:33, :], stB[64:65, :]

            with tc.tile_pool(name="xnp", bufs=1) as xnp:
                xn_sb = xnp.tile([128, 4, N], f32r)

                # ============ phase A: W transpose + stats + xn ============
                with (
                    tc.tile_pool(name="wnat", bufs=3) as wnat,
                    tc.tile_pool(name="tmp", bufs=2) as tmp,
                    tc.tile_pool(name="st_ps", bufs=1, space="PSUM") as st_ps,
                ):
                    st_px = st_ps.tile([1, N], f32, tag="sx")
                    st_pq = st_ps.tile([1, N], f32, tag="sq")

                    with tc.tile_pool(name="tp_ps", bufs=2, space="PSUM") as tp_ps:

                        def transp(dst, src):
                            ps = tp_ps.tile([128, 128], f32)
                            nc.tensor.transpose(ps, src, ident)
                            nc.scalar.copy(out=dst, in_=ps)

                        for ot in range(12):
                            wn = wnat.tile([128, C], f32, tag="wn")
                            nc.sync.dma_start(
                                out=wn, in_=wqkv_ap[ot * 128 : (ot + 1) * 128, :]
                            )
                            for kt in range(4):
                                src = wn[:, kt * 128 : (kt + 1) * 128]
                                if ot < 8:
                                    transp(
                                        wqkT[:, kt, ot * 128 : (ot + 1) * 128], src
                                    )
                                else:
                                    transp(
                                        wvT[:, kt, (ot - 8) * 128 : (ot - 7) * 128],
                                        src,
                                    )
                        for ot in range(4):
                            wn = wnat.tile([128, C], f32, tag="wn")
                            nc.sync.dma_start(
                                out=wn, in_=wout_ap[ot * 128 : (ot + 1) * 128, :]
                            )
                            for kt in range(4):
                                transp(
                                    woT[:, kt, ot * 128 : (ot + 1) * 128],
                                    wn[:, kt * 128 : (kt + 1) * 128],
                                )

                    # ---- stats rows: sum(x) fp32 matmul, sum(x^2) f32r ----
                    for t in range(4):
                        xsq = tmp.tile([128, N], f32r, tag="xsq")
                        nc.vector.tensor_mul(xsq, x_sb[:, t, :], x_sb[:, t, :])
                        for ch in range(2):
                            sl = slice(ch * 512, (ch + 1) * 512)
                            mm(
                                st_px[:, sl],
                                ones_f,
                                x_sb[:, t, sl],
                                start=(t == 0),
                                stop=(t == 3),
                            )
                            mm(
                                st_pq[:, sl],
                                ones_col,
                                xsq[:, sl],
                                start=(t == 0),
                                stop=(t == 3),
                            )

                    nc.scalar.activation(
                        mean_r, st_px, AF.Copy, scale=1.0 / C
                    )
                    nc.vector.tensor_mul(msq_r, mean_r, mean_r)
                    nc.vector.scalar_tensor_tensor(
                        out=var_r,
                        in0=st_pq,
                        scalar=1.0 / C,
                        in1=msq_r,
                        op0=OP.mult,
                        op1=OP.subtract,
                    )
                    nc.scalar.activation(
                        sd_r, var_r, AF.Sqrt, bias=eps_col[64:65, :]
                    )
                    nc.vector.reciprocal(a_r, sd_r)
                    nc.vector.scalar_tensor_tensor(
                        out=b_r,
                        in0=mean_r,
                        scalar=-1.0,
                        in1=a_r,
                        op0=OP.mult,
                        op1=OP.mult,
                    )
                    nc.scalar.copy(out=st_ra, in_=a_r)
                    nc.scalar.copy(out=st_rb, in_=b_r)

                    # ---- xn = x * (gamma (x) a) + (gamma (x) b) ----
                    ab_stack = tc.tile_pool(name="ab_ps", bufs=1, space="PSUM")
                    ab_ps = ab_stack.__enter__()
                    for t in range(4):
                        ab = ab_ps.tile([128, 2, N], f32)
                        tsl = slice(t * 128, (t + 1) * 128)
                        for ch in range(2):
                            sl = slice(ch * 512, (ch + 1) * 512)
                            mm(ab[:, 0, sl], gamma_row[0:1, tsl], st_ra[:, sl])
                            mm(ab[:, 1, sl], gamma_row[0:1, tsl], st_rb[:, sl])
                        t1 = tmp.tile([128, N], f32, tag="xnt")
                        nc.vector.tensor_mul(t1, x_sb[:, t, :], ab[:, 0, :])
                        nc.vector.tensor_add(xn_sb[:, t, :], t1, ab[:, 1, :])
                    ab_stack.__exit__(None, None, None)

                # ============ phase B: q,k and v^T projections ============
                with tc.tile_pool(name="qkv_ps", bufs=3, space="PSUM") as qkv_ps:
                    for ot in range(8):
                        for ch in range(2):
                            sl = slice(ch * 512, (ch + 1) * 512)
                            ps = qkv_ps.tile([128, 512], f32, tag="qk")
                            for kt in range(4):
                                mm(
                                    ps,
                                    wqkT[:, kt, ot * 128 : (ot + 1) * 128],
                                    xn_sb[:, kt, sl],
                                    start=(kt == 0),
                                    stop=(kt == 3),
                                )
                            nc.scalar.copy(out=qk_sb[:, ot, sl], in_=ps)

                    for nt in range(8):
                        ps = qkv_ps.tile([128, 512], f32, tag="vt")
                        for kt in range(4):
                            mm(
                                ps,
                                xn_sb[:, kt, nt * 128 : (nt + 1) * 128],
                                wvT[:, kt, :],
                                start=(kt == 0),
                                stop=(kt == 3),
                            )
                        for h in range(H):
                            nc.scalar.copy(
                                out=vT_sb[:, nt, h, 0:DH],
                                in_=ps[:, h * DH : (h + 1) * DH],
                            )

            # ================= phase C: attention =================
            with (
                tc.tile_pool(name="expT", bufs=1) as expp,
                tc.tile_pool(name="oan", bufs=2) as oan_p,
                tc.tile_pool(name="rdp", bufs=2) as rdp,
                tc.tile_pool(name="sim_ps", bufs=2, space="PSUM") as sim_ps,
                tc.tile_pool(name="oa_ps", bufs=1, space="PSUM") as oa_ps,
                tc.tile_pool(name="rb_ps", bufs=1, space="PSUM") as rb_ps,
            ):
                for h in range(H):
                    hp = (h % 2) * DH
                    q_h = qk_sb[hp : hp + DH, h // 2, :]
                    k_h = qk_sb[hp : hp + DH, 4 + h // 2, :]

                    expT = expp.tile([128, 8, N], f32r)
                    for jt in range(8):
                        sim = sim_ps.tile([128, N], f32)
                        for ch in range(2):
                            sl = slice(ch * 512, (ch + 1) * 512)
                            mm(
                                sim[:, sl],
                                k_h[:, jt * 128 : (jt + 1) * 128],
                                q_h[:, sl],
                            )
                        nc.scalar.activation(expT[:, jt, :], sim, AF.Exp, scale=SCALE)

                    oa = oa_ps.tile([DH + 1, N], f32)
                    for jt in range(8):
                        for ch in range(2):
                            sl = slice(ch * 512, (ch + 1) * 512)
                            mm(
                                oa[:, sl],
                                vT_sb[:, jt, h, :],
                                expT[:, jt, sl],
                                start=(jt == 0),
                                stop=(jt == 7),
                            )

                    nc.vector.reciprocal(rtmp, oa[DH : DH + 1, :])
                    rd = rdp.tile([1, N], f32r, tag="rd")
                    nc.scalar.copy(out=rd, in_=rtmp)
                    rb = rb_ps.tile([DH, N], f32)
                    for ch in range(2):
                        sl = slice(ch * 512, (ch + 1) * 512)
                        mm(rb[:, sl], ones_row, rd[:, sl])
                    # DVE reads at most one PSUM operand: stage numerator in SBUF
                    oan = oan_p.tile([DH, N], f32)
                    nc.scalar.copy(out=oan, in_=oa[0:DH, :])
                    nc.vector.tensor_mul(
                        att_sb[hp : hp + DH, h // 2, :], oan, rb
                    )

            # ============ phase D: out projection + residual ============
            with (
                tc.tile_pool(name="evict", bufs=3) as evict,
                tc.tile_pool(name="z_ps", bufs=3, space="PSUM") as z_ps,
            ):
                for ot in range(4):
                    for ch in range(2):
                        sl = slice(ch * 512, (ch + 1) * 512)
                        ps = z_ps.tile([128, 512], f32)
                        for kt in range(4):
                            mm(
                                ps,
                                woT[:, kt, ot * 128 : (ot + 1) * 128],
                                att_sb[:, kt, sl],
                                start=(kt == 0),
                                stop=(kt == 3),
                            )
                        yt = evict.tile([128, 512], f32)
                        nc.vector.tensor_add(yt, ps, x_sb[:, ot, sl])
                        nc.sync.dma_start(
                            out=y_ap[ot * 128 : (ot + 1) * 128, sl], in_=yt
                        )

    nc.compile()
    return nc


def _get_nc():
    if "nc" not in _CACHE:
        _CACHE["nc"] = _build()
    return _CACHE["nc"]


def kernel(x, gamma, w_qkv, w_out):
    from concourse.bass_utils import run_bass_kernel_spmd

    x = np.ascontiguousarray(x, dtype=np.float32)
    gamma_f = np.ascontiguousarray(gamma, dtype=np.float32).reshape(C)
    wq = np.ascontiguousarray(w_qkv, dtype=np.float32)
    wo = np.ascontiguousarray(w_out, dtype=np.float32)

    nc = _get_nc()
    in_maps = [
        {
            "x": x[b].reshape(C, N).copy(),
            "gamma": gamma_f,
            "w_qkv": wq,
            "w_out": wo,
        }
        for b in range(NCORES)
    ]
    res = run_bass_kernel_spmd(nc, in_maps, core_ids=list(range(NCORES)))
    out = np.stack(
        [res.results[b]["y"].reshape(C, 32, 32) for b in range(NCORES)], axis=0
    )
    return out.astype(np.float32)


# revision 20
# speedup vs baseline: 1.0331x; 1.0331x over previous
"""ConvAttention Trainium2 kernel.

Full-input contract: kernel(**inputs) takes the complete unsharded inputs
(x: (8, 512, 32, 32), gamma: (1, 512, 1, 1), w_qkv: (1536, 512),
w_out: (512, 512)) and returns the full (8, 512, 32, 32) output.

Sharding: data-parallel over batch — core b computes batch element b
entirely on-chip. No collectives.

Per-core math (b fixed), everything kept in [channel(part), spatial(free)]
layout:
  xn = LayerNorm_c(x) * gamma      (stats via ones-matmul rows on PE,
                                    row stats broadcast via K=1 matmuls)
  q,k = W_qk @ xn ; vT = xn^T W_v^T (computed directly transposed)
  per head: simT = k^T q; expT = exp(simT * s)  (softmax w/o max: logits
            are O(5), fp32 exp is safe)
  out_aug = [v;1]-augmented matmul -> numerator rows + denominator row
  attn_out = numerator * broadcast(1/denominator)
  y = w_out @ attn_out + x

Matmuls run in float32r (~1.5e-4 rel err, 4x fp32 throughput).
HW constraints honored: matmul operand/output base partitions and all AP
partition offsets are 32-aligned; f32r matmul inputs are produced by
compute ops with declared f32r outputs.
"""

import numpy as np

C = 512
N = 1024
O3 = 1536
H = 8
DH = 64
EPS = 1e-5
SCALE = 64.0 ** -0.5
NCORES = 8

_CACHE = {}


def _build():
    import concourse.bacc as bacc
    import concourse.tile as tile
    from concourse import mybir
    from concourse.masks import make_identity

    f32 = mybir.dt.float32
    f32r = mybir.dt.float32r
    AF = mybir.ActivationFunctionType
    OP = mybir.AluOpType

    nc = bacc.Bacc("TRN2", target_bir_lowering=False, debug=False, num_devices=1)
    x_ap = nc.dram_tensor("x", [C, N], f32, kind="ExternalInput").ap()
    g_ap = nc.dram_tensor("gamma", [C], f32, kind="ExternalInput").ap()
    wqkv_ap = nc.dram_tensor("w_qkv", [O3, C], f32, kind="ExternalInput").ap()
    wout_ap = nc.dram_tensor("w_out", [C, C], f32, kind="ExternalInput").ap()
    y_ap = nc.dram_tensor("y", [C, N], f32, kind="ExternalOutput").ap()

    mm = nc.tensor.matmul

    with tile.TileContext(nc) as tc:
        with (
            tc.tile_pool(name="const", bufs=1) as const,
            tc.tile_pool(name="xin", bufs=1) as xin,
            tc.tile_pool(name="acts", bufs=1) as acts,
            tc.tile_pool(name="wTp", bufs=1) as wTp,
            tc.tile_pool(name="rows", bufs=1) as rows,
        ):
            ident = const.tile([128, 128], f32)
            make_identity(nc, ident)
            ones_f = const.tile([128, 1], f32)
            nc.vector.memset(ones_f, 1.0)
            ones_col = const.tile([128, 1], f32r)
            nc.scalar.copy(out=ones_col, in_=ones_f)
            # row operands for K=1 broadcast matmuls (base partition 0 only)
            onesr_f = const.tile([1, DH], f32)
            nc.vector.memset(onesr_f, 1.0)
            ones_row = const.tile([1, DH], f32r)
            nc.scalar.copy(out=ones_row, in_=onesr_f)
            gamma_f = const.tile([1, C], f32)
            nc.sync.dma_start(out=gamma_f, in_=g_ap[None, :])
            gamma_row = const.tile([1, C], f32r)
            nc.scalar.copy(out=gamma_row, in_=gamma_f)
            vones_f = const.tile([128, 8, H, 1], f32)
            nc.vector.memset(vones_f, 1.0)
            eps_col = const.tile([65, 1], f32)
            nc.vector.memset(eps_col, EPS)

            # ---- load x ----
            x_sb = xin.tile([128, 4, N], f32)
            for t in range(4):
                nc.sync.dma_start(
                    out=x_sb[:, t, :], in_=x_ap[t * 128 : (t + 1) * 128, :]
                )

            # ---- persistent activation tiles ----
            qk_sb = acts.tile([128, 8, N], f32r)  # q: 0..3, k: 4..7
            vT_sb = acts.tile([128, 8, H, DH + 1], f32r)  # [nt, h, dh|ones]
            att_sb = acts.tile([128, 4, N], f32r)
            nc.scalar.copy(out=vT_sb[:, :, :, DH : DH + 1], in_=vones_f)

            # weights, transposed ([contraction-part, out-free])
            wqkT = wTp.tile([128, 4, 1024], f32r)
            wvT = wTp.tile([128, 4, C], f32r)
            woT = wTp.tile([128, 4, C], f32r)

            # stat rows, packed at 32-aligned partitions of shared tiles
            stA = rows.tile([97, N], f32)  # mean@0, msq@32, var@64, sd@96
            stB = rows.tile([97, N], f32)  # a@0, b@32, recip-tmp@64
            st_ra = rows.tile([1, N], f32r)  # a (rounded), base 0
            st_rb = rows.tile([1, N], f32r)  # b (rounded), base 0
            mean_r, msq_r, var_r, sd_r = (
                stA[0:1, :],
                stA[32:33, :],
                stA[64:65, :],
                stA[96:97, :],
            )
            a_r, b_r, rtmp = stB[0:1, :], stB[32:33, :], stB[64:65, :]

            with tc.tile_pool(name="xnp", bufs=1) as xnp:
                xn_sb = xnp.tile([128, 4, N], f32r)

                # ============ phase A: W transpose + stats + xn ============
                with (
                    tc.tile_pool(name="wnat", bufs=3) as wnat,
                    tc.tile_pool(name="tmp", bufs=2) as tmp,
                    tc.tile_pool(name="st_ps", bufs=1, space="PSUM") as st_ps,
                ):
                    st_px = st_ps.tile([1, N], f32, tag="sx")
                    st_pq = st_ps.tile([1, N], f32, tag="sq")

                    with tc.tile_pool(name="tp_ps", bufs=2, space="PSUM") as tp_ps:

                        def transp(dst, src):
                            ps = tp_ps.tile([128, 128], f32)
                            nc.tensor.transpose(ps, src, ident)
                            nc.vector.tensor_copy(dst, ps)

                        for ot in range(12):
                            wn = wnat.tile([128, C], f32, tag="wn")
                            nc.sync.dma_start(
                                out=wn, in_=wqkv_ap[ot * 128 : (ot + 1) * 128, :]
                            )
                            for kt in range(4):
                                src = wn[:, kt * 128 : (kt + 1) * 128]
                                if ot < 8:
                                    transp(
                                        wqkT[:, kt, ot * 128 : (ot + 1) * 128], src
                                    )
                                else:
                                    transp(
                                        wvT[:, kt, (ot - 8) * 128 : (ot - 7) * 128],
                                        src,
                                    )
                        for ot in range(4):
                            wn = wnat.tile([128, C], f32, tag="wn")
                            nc.sync.dma_start(
                                out=wn, in_=wout_ap[ot * 128 : (ot + 1) * 128, :]
                            )
                            for kt in range(4):
                                transp(
                                    woT[:, kt, ot * 128 : (ot + 1) * 128],
                                    wn[:, kt * 128 : (kt + 1) * 128],
                                )

                    # ---- stats rows: sum(x) fp32 matmul, sum(x^2) f32r ----
                    for t in range(4):
                        xsq = tmp.tile([128, N], f32r, tag="xsq")
                        nc.vector.tensor_mul(xsq, x_sb[:, t, :], x_sb[:, t, :])
                        for ch in range(2):
                            sl = slice(ch * 512, (ch + 1) * 512)
                            mm(
                                st_px[:, sl],
                                ones_f,
                                x_sb[:, t, sl],
                                start=(t == 0),
                                stop=(t == 3),
                            )
                            mm(
                                st_pq[:, sl],
                                ones_col,
                                xsq[:, sl],
                                start=(t == 0),
                                stop=(t == 3),
                            )

                    nc.scalar.activation(
                        mean_r, st_px, AF.Copy, scale=1.0 / C
                    )
                    nc.vector.tensor_mul(msq_r, mean_r, mean_r)
                    nc.vector.scalar_tensor_tensor(
                        out=var_r,
                        in0=st_pq,
                        scalar=1.0 / C,
                        in1=msq_r,
                        op0=OP.mult,
                        op1=OP.subtract,
                    )
                    nc.scalar.activation(
                        sd_r, var_r, AF.Sqrt, bias=eps_col[64:65, :]
                    )
                    nc.vector.reciprocal(a_r, sd_r)
                    nc.vector.scalar_tensor_tensor(
                        out=b_r,
                        in0=mean_r,
                        scalar=-1.0,
                        in1=a_r,
                        op0=OP.mult,
                        op1=OP.mult,
                    )
                    nc.scalar.copy(out=st_ra, in_=a_r)
                    nc.scalar.copy(out=st_rb, in_=b_r)

                    # ---- xn = x * (gamma (x) a) + (gamma (x) b) ----
                    ab_stack = tc.tile_pool(name="ab_ps", bufs=1, space="PSUM")
                    ab_ps = ab_stack.__enter__()
                    for t in range(4):
                        ab = ab_ps.tile([128, 2, N], f32)
                        tsl = slice(t * 128, (t + 1) * 128)
                        for ch in range(2):
                            sl = slice(ch * 512, (ch + 1) * 512)
                            mm(ab[:, 0, sl], gamma_row[0:1, tsl], st_ra[:, sl])
                            mm(ab[:, 1, sl], gamma_row[0:1, tsl], st_rb[:, sl])
                        t1 = tmp.tile([128, N], f32, tag="xnt")
                        nc.vector.tensor_mul(t1, x_sb[:, t, :], ab[:, 0, :])
                        nc.vector.tensor_add(xn_sb[:, t, :], t1, ab[:, 1, :])
                    ab_stack.__exit__(None, None, None)

                # ============ phase B: q,k and v^T projections ============
                with (
                    tc.tile_pool(name="qk_ps", bufs=2, space="PSUM") as qk_psp,
                    tc.tile_pool(name="vt_ps", bufs=3, space="PSUM") as vt_psp,
                ):
                    for ot in (0, 4, 1, 5, 2, 6, 3, 7):
                        ps = qk_psp.tile([128, 2, 512], f32, tag="qk")
                        for kt in range(4):
                            for ch in range(2):
                                sl = slice(ch * 512, (ch + 1) * 512)
                                mm(
                                    ps[:, ch, :],
                                    wqkT[:, kt, ot * 128 : (ot + 1) * 128],
                                    xn_sb[:, kt, sl],
                                    start=(kt == 0),
                                    stop=(kt == 3),
                                )
                        nc.scalar.copy(out=qk_sb[:, ot, :], in_=ps)

                    for nt in range(8):
                        ps = vt_psp.tile([128, 512], f32, tag="vt")
                        for kt in range(4):
                            mm(
                                ps,
                                xn_sb[:, kt, nt * 128 : (nt + 1) * 128],
                                wvT[:, kt, :],
                                start=(kt == 0),
                                stop=(kt == 3),
                            )
                        for h in range(H):
                            nc.vector.tensor_copy(
                                vT_sb[:, nt, h, 0:DH],
                                ps[:, h * DH : (h + 1) * DH],
                            )

            # ================= phase C: attention =================
            with (
                tc.tile_pool(name="expT", bufs=6) as expp,
                tc.tile_pool(name="oan", bufs=2) as oan_p,
                tc.tile_pool(name="rdp", bufs=2) as rdp,
                tc.tile_pool(name="sim_ps", bufs=2, space="PSUM") as sim_ps,
                tc.tile_pool(name="oa_ps", bufs=1, space="PSUM") as oa_ps,
                tc.tile_pool(name="rb_ps", bufs=1, space="PSUM") as rb_ps,
            ):
                for h in range(H):
                    hp = (h % 2) * DH
                    q_h = qk_sb[hp : hp + DH, h // 2, :]
                    k_h = qk_sb[hp : hp + DH, 4 + h // 2, :]

                    expTs = []
                    for jt in range(8):
                        sim = sim_ps.tile([128, N], f32)
                        for ch in range(2):
                            sl = slice(ch * 512, (ch + 1) * 512)
                            mm(
                                sim[:, sl],
                                k_h[:, jt * 128 : (jt + 1) * 128],
                                q_h[:, sl],
                            )
                        expT = expp.tile([128, N], f32r, tag="e")
                        nc.scalar.activation(expT, sim, AF.Exp, scale=SCALE)
                        expTs.append(expT)

                    oa = oa_ps.tile([DH + 1, N], f32)
                    for jt in range(8):
                        for ch in range(2):
                            sl = slice(ch * 512, (ch + 1) * 512)
                            mm(
                                oa[:, sl],
                                vT_sb[:, jt, h, :],
                                expTs[jt][:, sl],
                                start=(jt == 0),
                                stop=(jt == 7),
                            )

                    nc.vector.reciprocal(rtmp, oa[DH : DH + 1, :])
                    rd = rdp.tile([1, N], f32r, tag="rd")
                    nc.scalar.copy(out=rd, in_=rtmp)
                    rb = rb_ps.tile([DH, N], f32)
                    for ch in range(2):
                        sl = slice(ch * 512, (ch + 1) * 512)
                        mm(rb[:, sl], ones_row, rd[:, sl])
                    # DVE reads at most one PSUM operand: stage numerator in SBUF
                    oan = oan_p.tile([DH, N], f32)
                    nc.vector.tensor_copy(oan, oa[0:DH, :])
                    nc.vector.tensor_mul(
                        att_sb[hp : hp + DH, h // 2, :], oan, rb
                    )

            # ============ phase D: out projection + residual ============
            with (
                tc.tile_pool(name="evict", bufs=3) as evict,
                tc.tile_pool(name="z_ps", bufs=3, space="PSUM") as z_ps,
            ):
                for ot in range(4):
                    for ch in range(2):
                        sl = slice(ch * 512, (ch + 1) * 512)
                        ps = z_ps.tile([128, 512], f32)
                        for kt in range(4):
                            mm(
                                ps,
                                woT[:, kt, ot * 128 : (ot + 1) * 128],
                                att_sb[:, kt, sl],
                                start=(kt == 0),
                                stop=(kt == 3),
                            )
                        yt = evict.tile([128, 512], f32)
                        nc.vector.tensor_add(yt, ps, x_sb[:, ot, sl])
                        nc.sync.dma_start(
                            out=y_ap[ot * 128 : (ot + 1) * 128, sl], in_=yt
                        )

    nc.compile()
    return nc


def _get_nc():
    if "nc" not in _CACHE:
        _CACHE["nc"] = _build()
    return _CACHE["nc"]


def kernel(x, gamma, w_qkv, w_out):
    from concourse.bass_utils import run_bass_kernel_spmd

    x = np.ascontiguousarray(x, dtype=np.float32)
    gamma_f = np.ascontiguousarray(gamma, dtype=np.float32).reshape(C)
    wq = np.ascontiguousarray(w_qkv, dtype=np.float32)
    wo = np.ascontiguousarray(w_out, dtype=np.float32)

    nc = _get_nc()
    in_maps = [
        {
            "x": x[b].reshape(C, N).copy(),
            "gamma": gamma_f,
            "w_qkv": wq,
            "w_out": wo,
        }
        for b in range(NCORES)
    ]
    res = run_bass_kernel_spmd(nc, in_maps, core_ids=list(range(NCORES)))
    out = np.stack(
        [res.results[b]["y"].reshape(C, 32, 32) for b in range(NCORES)], axis=0
    )
    return out.astype(np.float32)


# revision 21
# speedup vs baseline: 1.1974x; 1.1591x over previous
"""ConvAttention Trainium2 kernel.

Full-input contract: kernel(**inputs) takes the complete unsharded inputs
(x: (8, 512, 32, 32), gamma: (1, 512, 1, 1), w_qkv: (1536, 512),
w_out: (512, 512)) and returns the full (8, 512, 32, 32) output.

Sharding: data-parallel over batch — core b computes batch element b
entirely on-chip. No collectives.

Per-core math (b fixed), everything kept in [channel(part), spatial(free)]
layout:
  xn = LayerNorm_c(x) * gamma      (stats via ones-matmul rows on PE,
                                    row stats broadcast via K=1 matmuls)
  q,k = W_qk @ xn ; vT = xn^T W_v^T (computed directly transposed)
  per head: simT = k^T q; expT = exp(simT * s)  (softmax w/o max: logits
            are O(5), fp32 exp is safe)
  out_aug = [v;1]-augmented matmul -> numerator rows + denominator row
  attn_out = numerator * broadcast(1/denominator)
  y = w_out @ attn_out + x

Matmuls run in float32r (~1.5e-4 rel err, 4x fp32 throughput).
HW constraints honored: matmul operand/output base partitions and all AP
partition offsets are 32-aligned; f32r matmul inputs are produced by
compute ops with declared f32r outputs.
"""

import numpy as np

C = 512
N = 1024
O3 = 1536
H = 8
DH = 64
EPS = 1e-5
SCALE = 64.0 ** -0.5
NCORES = 8

_CACHE = {}


def _build():
    import concourse.bacc as bacc
    import concourse.tile as tile
    from concourse import mybir
    from concourse.masks import make_identity

    f32 = mybir.dt.float32
    f32r = mybir.dt.float32r
    bf16 = mybir.dt.bfloat16
    AF = mybir.ActivationFunctionType
    OP = mybir.AluOpType

    nc = bacc.Bacc("TRN2", target_bir_lowering=False, debug=False, num_devices=1)
    x_ap = nc.dram_tensor("x", [C, N], f32, kind="ExternalInput").ap()
    g_ap = nc.dram_tensor("gamma", [C], f32, kind="ExternalInput").ap()
    wqkv_ap = nc.dram_tensor("w_qkv", [O3, C], f32, kind="ExternalInput").ap()
    wout_ap = nc.dram_tensor("w_out", [C, C], f32, kind="ExternalInput").ap()
    y_ap = nc.dram_tensor("y", [C, N], f32, kind="ExternalOutput").ap()

    mm = nc.tensor.matmul

    with tile.TileContext(nc) as tc:
        with (
            tc.tile_pool(name="const", bufs=1) as const,
            tc.tile_pool(name="xin", bufs=1) as xin,
            tc.tile_pool(name="acts", bufs=1) as acts,
            tc.tile_pool(name="wTp", bufs=1) as wTp,
            tc.tile_pool(name="rows", bufs=1) as rows,
        ):
            ident = const.tile([128, 128], f32)
            make_identity(nc, ident)
            ones_f = const.tile([128, 1], f32)
            nc.vector.memset(ones_f, 1.0)
            ones_col = const.tile([128, 1], f32r)
            nc.scalar.copy(out=ones_col, in_=ones_f)
            # row operands for K=1 broadcast matmuls (base partition 0 only)
            onesr_f = const.tile([1, DH], f32)
            nc.vector.memset(onesr_f, 1.0)
            ones_row = const.tile([1, DH], f32r)
            nc.scalar.copy(out=ones_row, in_=onesr_f)
            gamma_f = const.tile([1, C], f32)
            nc.sync.dma_start(out=gamma_f, in_=g_ap[None, :])
            gamma_row = const.tile([1, C], f32r)
            nc.scalar.copy(out=gamma_row, in_=gamma_f)
            vones_f = const.tile([128, 8, H, 1], f32)
            nc.vector.memset(vones_f, 1.0)
            eps_col = const.tile([65, 1], f32)
            nc.vector.memset(eps_col, EPS)

            # ---- load x ----
            x_sb = xin.tile([128, 4, N], f32)
            for t in range(4):
                nc.sync.dma_start(
                    out=x_sb[:, t, :], in_=x_ap[t * 128 : (t + 1) * 128, :]
                )

            # ---- persistent activation tiles ----
            qk_sb = acts.tile([128, 8, N], bf16)  # q: 0..3, k: 4..7
            vT_sb = acts.tile([128, 8, H, DH + 1], bf16)  # [nt, h, dh|ones]
            att_sb = acts.tile([128, 4, N], bf16)
            nc.scalar.copy(out=vT_sb[:, :, :, DH : DH + 1], in_=vones_f)

            # weights, transposed ([contraction-part, out-free])
            wqkT = wTp.tile([128, 4, 1024], bf16)
            wvT = wTp.tile([128, 4, C], bf16)
            woT = wTp.tile([128, 4, C], bf16)

            # stat rows, packed at 32-aligned partitions of shared tiles
            stA = rows.tile([97, N], f32)  # mean@0, msq@32, var@64, sd@96
            stB = rows.tile([97, N], f32)  # a@0, b@32, recip-tmp@64
            st_ra = rows.tile([1, N], f32r)  # a (rounded), base 0
            st_rb = rows.tile([1, N], f32r)  # b (rounded), base 0
            mean_r, msq_r, var_r, sd_r = (
                stA[0:1, :],
                stA[32:33, :],
                stA[64:65, :],
                stA[96:97, :],
            )
            a_r, b_r, rtmp = stB[0:1, :], stB[32:33, :], stB[64:65, :]
            rden = rows.tile([1, N], f32)  # denom staged at partition 0
            rrec = rows.tile([1, N], f32)  # 1/denom (fast approx)

            with tc.tile_pool(name="xnp", bufs=1) as xnp:
                xn_sb = xnp.tile([128, 4, N], bf16)

                # ============ phase A: W transpose + stats + xn ============
                with (
                    tc.tile_pool(name="wnat", bufs=3) as wnat,
                    tc.tile_pool(name="tmp", bufs=2) as tmp,
                    tc.tile_pool(name="st_ps", bufs=1, space="PSUM") as st_ps,
                ):
                    st_px = st_ps.tile([1, N], f32, tag="sx")
                    st_pq = st_ps.tile([1, N], f32, tag="sq")

                    with tc.tile_pool(name="tp_ps", bufs=2, space="PSUM") as tp_ps:

                        def transp(dst, src):
                            ps = tp_ps.tile([128, 128], f32)
                            nc.tensor.transpose(ps, src, ident)
                            nc.vector.tensor_copy(dst, ps)

                        for ot in range(12):
                            wn = wnat.tile([128, C], f32, tag="wn")
                            nc.sync.dma_start(
                                out=wn, in_=wqkv_ap[ot * 128 : (ot + 1) * 128, :]
                            )
                            for kt in range(4):
                                src = wn[:, kt * 128 : (kt + 1) * 128]
                                if ot < 8:
                                    transp(
                                        wqkT[:, kt, ot * 128 : (ot + 1) * 128], src
                                    )
                                else:
                                    transp(
                                        wvT[:, kt, (ot - 8) * 128 : (ot - 7) * 128],
                                        src,
                                    )
                        for ot in range(4):
                            wn = wnat.tile([128, C], f32, tag="wn")
                            nc.sync.dma_start(
                                out=wn, in_=wout_ap[ot * 128 : (ot + 1) * 128, :]
                            )
                            for kt in range(4):
                                transp(
                                    woT[:, kt, ot * 128 : (ot + 1) * 128],
                                    wn[:, kt * 128 : (kt + 1) * 128],
                                )

                    # ---- stats rows: sum(x) fp32 matmul, sum(x^2) f32r ----
                    for t in range(4):
                        xsq = tmp.tile([128, N], f32r, tag="xsq")
                        nc.vector.tensor_mul(xsq, x_sb[:, t, :], x_sb[:, t, :])
                        for ch in range(2):
                            sl = slice(ch * 512, (ch + 1) * 512)
                            mm(
                                st_px[:, sl],
                                ones_f,
                                x_sb[:, t, sl],
                                start=(t == 0),
                                stop=(t == 3),
                            )
                            mm(
                                st_pq[:, sl],
                                ones_col,
                                xsq[:, sl],
                                start=(t == 0),
                                stop=(t == 3),
                            )

                    nc.scalar.activation(
                        mean_r, st_px, AF.Copy, scale=1.0 / C
                    )
                    nc.vector.tensor_mul(msq_r, mean_r, mean_r)
                    nc.vector.scalar_tensor_tensor(
                        out=var_r,
                        in0=st_pq,
                        scalar=1.0 / C,
                        in1=msq_r,
                        op0=OP.mult,
                        op1=OP.subtract,
                    )
                    nc.scalar.activation(
                        sd_r, var_r, AF.Sqrt, bias=eps_col[64:65, :]
                    )
                    nc.vector.reciprocal(a_r, sd_r)
                    nc.vector.scalar_tensor_tensor(
                        out=b_r,
                        in0=mean_r,
                        scalar=-1.0,
                        in1=a_r,
                        op0=OP.mult,
                        op1=OP.mult,
                    )
                    nc.scalar.copy(out=st_ra, in_=a_r)
                    nc.scalar.copy(out=st_rb, in_=b_r)

                    # ---- xn = x * (gamma (x) a) + (gamma (x) b) ----
                    ab_stack = tc.tile_pool(name="ab_ps", bufs=1, space="PSUM")
                    ab_ps = ab_stack.__enter__()
                    for t in range(4):
                        ab = ab_ps.tile([128, 2, N], f32)
                        tsl = slice(t * 128, (t + 1) * 128)
                        for ch in range(2):
                            sl = slice(ch * 512, (ch + 1) * 512)
                            mm(ab[:, 0, sl], gamma_row[0:1, tsl], st_ra[:, sl])
                            mm(ab[:, 1, sl], gamma_row[0:1, tsl], st_rb[:, sl])
                        t1 = tmp.tile([128, N], f32, tag="xnt")
                        nc.vector.tensor_mul(t1, x_sb[:, t, :], ab[:, 0, :])
                        nc.vector.tensor_add(xn_sb[:, t, :], t1, ab[:, 1, :])
                    ab_stack.__exit__(None, None, None)

                # ============ phase B: q,k and v^T projections ============
                with (
                    tc.tile_pool(name="qk_ps", bufs=2, space="PSUM") as qk_psp,
                    tc.tile_pool(name="vt_ps", bufs=3, space="PSUM") as vt_psp,
                ):
                    for ot in (0, 4, 1, 5, 2, 6, 3, 7):
                        ps = qk_psp.tile([128, 2, 512], f32, tag="qk")
                        for kt in range(4):
                            for ch in range(2):
                                sl = slice(ch * 512, (ch + 1) * 512)
                                mm(
                                    ps[:, ch, :],
                                    wqkT[:, kt, ot * 128 : (ot + 1) * 128],
                                    xn_sb[:, kt, sl],
                                    start=(kt == 0),
                                    stop=(kt == 3),
                                )
                        nc.scalar.copy(out=qk_sb[:, ot, :], in_=ps)

                    for nt in range(8):
                        ps = vt_psp.tile([128, 512], f32, tag="vt")
                        for kt in range(4):
                            mm(
                                ps,
                                xn_sb[:, kt, nt * 128 : (nt + 1) * 128],
                                wvT[:, kt, :],
                                start=(kt == 0),
                                stop=(kt == 3),
                            )
                        for h in range(H):
                            nc.vector.tensor_copy(
                                vT_sb[:, nt, h, 0:DH],
                                ps[:, h * DH : (h + 1) * DH],
                            )

            # ================= phase C: attention =================
            with (
                tc.tile_pool(name="expT", bufs=6) as expp,
                tc.tile_pool(name="oan", bufs=2) as oan_p,
                tc.tile_pool(name="rdp", bufs=2) as rdp,
                tc.tile_pool(name="sim_ps", bufs=2, space="PSUM") as sim_ps,
                tc.tile_pool(name="oa_ps", bufs=1, space="PSUM") as oa_ps,
                tc.tile_pool(name="rb_ps", bufs=1, space="PSUM") as rb_ps,
            ):
                for h in range(H):
                    hp = (h % 2) * DH
                    q_h = qk_sb[hp : hp + DH, h // 2, :]
                    k_h = qk_sb[hp : hp + DH, 4 + h // 2, :]

                    expTs = []
                    for jt in range(8):
                        sim = sim_ps.tile([128, N], f32)
                        for ch in range(2):
                            sl = slice(ch * 512, (ch + 1) * 512)
                            mm(
                                sim[:, sl],
                                k_h[:, jt * 128 : (jt + 1) * 128],
                                q_h[:, sl],
                            )
                        expT = expp.tile([128, N], bf16, tag="e")
                        nc.scalar.activation(expT, sim, AF.Exp, scale=SCALE)
                        expTs.append(expT)

                    oa = oa_ps.tile([DH + 1, N], f32)
                    for jt in range(8):
                        for ch in range(2):
                            sl = slice(ch * 512, (ch + 1) * 512)
                            mm(
                                oa[:, sl],
                                vT_sb[:, jt, h, :],
                                expTs[jt][:, sl],
                                start=(jt == 0),
                                stop=(jt == 7),
                            )

                    nc.scalar.copy(out=rden, in_=oa[DH : DH + 1, :])
                    nc.vector.reciprocal_approx_fast(out=rrec, in_=rden)
                    rd = rdp.tile([1, N], f32r, tag="rd")
                    nc.scalar.copy(out=rd, in_=rrec)
                    rb = rb_ps.tile([DH, N], f32)
                    for ch in range(2):
                        sl = slice(ch * 512, (ch + 1) * 512)
                        mm(rb[:, sl], ones_row, rd[:, sl])
                    # DVE reads at most one PSUM operand: stage numerator in SBUF
                    oan = oan_p.tile([DH, N], f32)
                    nc.vector.tensor_copy(oan, oa[0:DH, :])
                    nc.vector.tensor_mul(
                        att_sb[hp : hp + DH, h // 2, :], oan, rb
                    )

            # ============ phase D: out projection + residual ============
            with (
                tc.tile_pool(name="evict", bufs=3) as evict,
                tc.tile_pool(name="z_ps", bufs=3, space="PSUM") as z_ps,
            ):
                for ot in range(4):
                    for ch in range(2):
                        sl = slice(ch * 512, (ch + 1) * 512)
                        ps = z_ps.tile([128, 512], f32)
                        for kt in range(4):
                            mm(
                                ps,
                                woT[:, kt, ot * 128 : (ot + 1) * 128],
                                att_sb[:, kt, sl],
                                start=(kt == 0),
                                stop=(kt == 3),
                            )
                        yt = evict.tile([128, 512], f32)
                        nc.vector.tensor_add(yt, ps, x_sb[:, ot, sl])
                        nc.sync.dma_start(
                            out=y_ap[ot * 128 : (ot + 1) * 128, sl], in_=yt
                        )

    nc.compile()
    return nc


def _get_nc():
    if "nc" not in _CACHE:
        _CACHE["nc"] = _build()
    return _CACHE["nc"]


def kernel(x, gamma, w_qkv, w_out):
    from concourse.bass_utils import run_bass_kernel_spmd

    x = np.ascontiguousarray(x, dtype=np.float32)
    gamma_f = np.ascontiguousarray(gamma, dtype=np.float32).reshape(C)
    wq = np.ascontiguousarray(w_qkv, dtype=np.float32)
    wo = np.ascontiguousarray(w_out, dtype=np.float32)

    nc = _get_nc()
    in_maps = [
        {
            "x": x[b].reshape(C, N).copy(),
            "gamma": gamma_f,
            "w_qkv": wq,
            "w_out": wo,
        }
        for b in range(NCORES)
    ]
    res = run_bass_kernel_spmd(nc, in_maps, core_ids=list(range(NCORES)))
    out = np.stack(
        [res.results[b]["y"].reshape(C, 32, 32) for b in range(NCORES)], axis=0
    )
    return out.astype(np.float32)


# revision 22
# speedup vs baseline: 1.3634x; 1.1386x over previous
"""ConvAttention Trainium2 kernel.

Full-input contract: kernel(**inputs) takes the complete unsharded inputs
(x: (8, 512, 32, 32), gamma: (1, 512, 1, 1), w_qkv: (1536, 512),
w_out: (512, 512)) and returns the full (8, 512, 32, 32) output.

Sharding: data-parallel over batch — core b computes batch element b
entirely on-chip. No collectives.

Per-core math (b fixed), everything kept in [channel(part), spatial(free)]
layout:
  xn = LayerNorm_c(x) * gamma      (stats via ones-matmul rows on PE,
                                    row stats broadcast via K=1 matmuls)
  q,k = W_qk @ xn ; vT = xn^T W_v^T (computed directly transposed)
  per head: simT = k^T q; expT = exp(simT * s)  (softmax w/o max: logits
            are O(5), fp32 exp is safe)
  out_aug = [v;1]-augmented matmul -> numerator rows + denominator row
  attn_out = numerator * broadcast(1/denominator)
  y = w_out @ attn_out + x

Matmuls run in float32r (~1.5e-4 rel err, 4x fp32 throughput).
HW constraints honored: matmul operand/output base partitions and all AP
partition offsets are 32-aligned; f32r matmul inputs are produced by
compute ops with declared f32r outputs.
"""

import numpy as np

C = 512
N = 1024
O3 = 1536
H = 8
DH = 64
EPS = 1e-5
SCALE = 64.0 ** -0.5
NCORES = 8

_CACHE = {}


def _build():
    import concourse.bacc as bacc
    import concourse.tile as tile
    from concourse import mybir
    from concourse.masks import make_identity

    f32 = mybir.dt.float32
    f32r = mybir.dt.float32r
    bf16 = mybir.dt.bfloat16
    AF = mybir.ActivationFunctionType
    OP = mybir.AluOpType

    nc = bacc.Bacc("TRN2", target_bir_lowering=False, debug=False, num_devices=1)
    x_ap = nc.dram_tensor("x", [C, N], f32, kind="ExternalInput").ap()
    g_ap = nc.dram_tensor("gamma", [C], f32, kind="ExternalInput").ap()
    wqkv_ap = nc.dram_tensor("w_qkv", [O3, C], f32, kind="ExternalInput").ap()
    wout_ap = nc.dram_tensor("w_out", [C, C], f32, kind="ExternalInput").ap()
    y_ap = nc.dram_tensor("y", [C, N], f32, kind="ExternalOutput").ap()

    mm = nc.tensor.matmul

    with tile.TileContext(nc) as tc:
        with (
            tc.tile_pool(name="const", bufs=1) as const,
            tc.tile_pool(name="xin", bufs=1) as xin,
            tc.tile_pool(name="acts", bufs=1) as acts,
            tc.tile_pool(name="wTp", bufs=1) as wTp,
            tc.tile_pool(name="rows", bufs=1) as rows,
        ):
            ident = const.tile([128, 128], f32)
            make_identity(nc, ident)
            ones_f = const.tile([128, 1], f32)
            nc.vector.memset(ones_f, 1.0)
            ones_col = const.tile([128, 1], f32r)
            nc.scalar.copy(out=ones_col, in_=ones_f)
            # row operands for K=1 broadcast matmuls (base partition 0 only)
            onesr_f = const.tile([1, DH], f32)
            nc.vector.memset(onesr_f, 1.0)
            ones_row = const.tile([1, DH], f32r)
            nc.scalar.copy(out=ones_row, in_=onesr_f)
            gamma_f = const.tile([1, C], f32)
            nc.sync.dma_start(out=gamma_f, in_=g_ap[None, :])
            gamma_row = const.tile([1, C], f32r)
            nc.scalar.copy(out=gamma_row, in_=gamma_f)
            vones_f = const.tile([128, 8, H, 1], f32)
            nc.vector.memset(vones_f, 1.0)
            eps_col = const.tile([65, 1], f32)
            nc.vector.memset(eps_col, EPS)

            # ---- load x ----
            x_sb = xin.tile([128, 4, N], f32)
            for t in range(4):
                nc.sync.dma_start(
                    out=x_sb[:, t, :], in_=x_ap[t * 128 : (t + 1) * 128, :]
                )

            # ---- persistent activation tiles ----
            qk_sb = acts.tile([128, 8, N], bf16)  # q: 0..3, k: 4..7
            vT_sb = acts.tile([128, 8, H, DH + 1], bf16)  # [nt, h, dh|ones]
            att_sb = acts.tile([128, 4, N], bf16)
            nc.scalar.copy(out=vT_sb[:, :, :, DH : DH + 1], in_=vones_f)

            # weights, transposed ([contraction-part, out-free])
            wqkT = wTp.tile([128, 4, 1024], bf16)
            wvT = wTp.tile([128, 4, C], bf16)
            woT = wTp.tile([128, 4, C], bf16)

            # stat rows, packed at 32-aligned partitions of shared tiles
            stA = rows.tile([97, N], f32)  # mean@0, msq@32, var@64, sd@96
            stB = rows.tile([97, N], f32)  # a@0, b@32, recip-tmp@64
            st_ra = rows.tile([1, N], f32r)  # a (rounded), base 0
            st_rb = rows.tile([1, N], f32r)  # b (rounded), base 0
            mean_r, msq_r, var_r, sd_r = (
                stA[0:1, :],
                stA[32:33, :],
                stA[64:65, :],
                stA[96:97, :],
            )
            a_r, b_r, rtmp = stB[0:1, :], stB[32:33, :], stB[64:65, :]
            rden = rows.tile([1, N], f32)  # denom staged at partition 0
            rrec = rows.tile([1, N], f32)  # 1/denom (fast approx)

            with tc.tile_pool(name="xnp", bufs=1) as xnp:
                xn_sb = xnp.tile([128, 4, N], bf16)

                # ============ phase A: W transpose + stats + xn ============
                with (
                    tc.tile_pool(name="wnat", bufs=3) as wnat,
                    tc.tile_pool(name="tmp", bufs=2) as tmp,
                    tc.tile_pool(name="st_ps", bufs=1, space="PSUM") as st_ps,
                ):
                    st_px = st_ps.tile([1, N], f32, tag="sx")
                    st_pq = st_ps.tile([1, N], f32, tag="sq")

                    # ---- stats rows: sum(x) fp32 matmul, sum(x^2) f32r ----
                    for t in range(4):
                        xsq = tmp.tile([128, N], f32r, tag="xsq")
                        nc.vector.tensor_mul(xsq, x_sb[:, t, :], x_sb[:, t, :])
                        for ch in range(2):
                            sl = slice(ch * 512, (ch + 1) * 512)
                            mm(
                                st_px[:, sl],
                                ones_f,
                                x_sb[:, t, sl],
                                start=(t == 0),
                                stop=(t == 3),
                            )
                            mm(
                                st_pq[:, sl],
                                ones_col,
                                xsq[:, sl],
                                start=(t == 0),
                                stop=(t == 3),
                            )

                    with tc.tile_pool(name="tp_ps", bufs=2, space="PSUM") as tp_ps:

                        def transp(dst, src):
                            ps = tp_ps.tile([128, 128], f32)
                            nc.tensor.transpose(ps, src, ident)
                            nc.vector.tensor_copy(dst, ps)

                        for ot in range(12):
                            wn = wnat.tile([128, C], f32, tag="wn")
                            nc.sync.dma_start(
                                out=wn, in_=wqkv_ap[ot * 128 : (ot + 1) * 128, :]
                            )
                            for kt in range(4):
                                src = wn[:, kt * 128 : (kt + 1) * 128]
                                if ot < 8:
                                    transp(
                                        wqkT[:, kt, ot * 128 : (ot + 1) * 128], src
                                    )
                                else:
                                    transp(
                                        wvT[:, kt, (ot - 8) * 128 : (ot - 7) * 128],
                                        src,
                                    )
                        for ot in range(4):
                            wn = wnat.tile([128, C], f32, tag="wn")
                            nc.sync.dma_start(
                                out=wn, in_=wout_ap[ot * 128 : (ot + 1) * 128, :]
                            )
                            for kt in range(4):
                                transp(
                                    woT[:, kt, ot * 128 : (ot + 1) * 128],
                                    wn[:, kt * 128 : (kt + 1) * 128],
                                )

                    nc.scalar.activation(
                        mean_r, st_px, AF.Copy, scale=1.0 / C
                    )
                    nc.vector.tensor_mul(msq_r, mean_r, mean_r)
                    nc.vector.scalar_tensor_tensor(
                        out=var_r,
                        in0=st_pq,
                        scalar=1.0 / C,
                        in1=msq_r,
                        op0=OP.mult,
                        op1=OP.subtract,
                    )
                    nc.scalar.activation(
                        sd_r, var_r, AF.Sqrt, bias=eps_col[64:65, :]
                    )
                    nc.vector.reciprocal(a_r, sd_r)
                    nc.vector.scalar_tensor_tensor(
                        out=b_r,
                        in0=mean_r,
                        scalar=-1.0,
                        in1=a_r,
                        op0=OP.mult,
                        op1=OP.mult,
                    )
                    nc.scalar.copy(out=st_ra, in_=a_r)
                    nc.scalar.copy(out=st_rb, in_=b_r)

                    # ---- xn = x * (gamma (x) a) + (gamma (x) b) ----
                    ab_stack = tc.tile_pool(name="ab_ps", bufs=1, space="PSUM")
                    ab_ps = ab_stack.__enter__()
                    for t in range(4):
                        ab = ab_ps.tile([128, 2, N], f32)
                        tsl = slice(t * 128, (t + 1) * 128)
                        for ch in range(2):
                            sl = slice(ch * 512, (ch + 1) * 512)
                            mm(ab[:, 0, sl], gamma_row[0:1, tsl], st_ra[:, sl])
                            mm(ab[:, 1, sl], gamma_row[0:1, tsl], st_rb[:, sl])
                        t1 = tmp.tile([128, N], f32, tag="xnt")
                        nc.vector.tensor_mul(t1, x_sb[:, t, :], ab[:, 0, :])
                        nc.vector.tensor_add(xn_sb[:, t, :], t1, ab[:, 1, :])
                    ab_stack.__exit__(None, None, None)

                # ============ phase B: q,k and v^T projections ============
                with (
                    tc.tile_pool(name="qk_ps", bufs=2, space="PSUM") as qk_psp,
                    tc.tile_pool(name="vt_ps", bufs=3, space="PSUM") as vt_psp,
                ):
                    for ot in (0, 4, 1, 5, 2, 6, 3, 7):
                        ps = qk_psp.tile([128, 2, 512], f32, tag="qk")
                        for kt in range(4):
                            for ch in range(2):
                                sl = slice(ch * 512, (ch + 1) * 512)
                                mm(
                                    ps[:, ch, :],
                                    wqkT[:, kt, ot * 128 : (ot + 1) * 128],
                                    xn_sb[:, kt, sl],
                                    start=(kt == 0),
                                    stop=(kt == 3),
                                )
                        nc.scalar.copy(out=qk_sb[:, ot, :], in_=ps)

                    for nt in range(8):
                        ps = vt_psp.tile([128, 512], f32, tag="vt")
                        for kt in range(4):
                            mm(
                                ps,
                                xn_sb[:, kt, nt * 128 : (nt + 1) * 128],
                                wvT[:, kt, :],
                                start=(kt == 0),
                                stop=(kt == 3),
                            )
                        nc.vector.tensor_copy(
                            vT_sb[:, nt, :, 0:DH],
                            ps.rearrange("p (h d) -> p h d", h=H),
                        )

            # ================= phase C: attention =================
            with (
                tc.tile_pool(name="expT", bufs=6) as expp,
                tc.tile_pool(name="oan", bufs=2) as oan_p,
                tc.tile_pool(name="rdp", bufs=2) as rdp,
                tc.tile_pool(name="sim_ps", bufs=2, space="PSUM") as sim_ps,
                tc.tile_pool(name="oa_ps", bufs=1, space="PSUM") as oa_ps,
                tc.tile_pool(name="rb_ps", bufs=1, space="PSUM") as rb_ps,
            ):
                for h in range(H):
                    hp = (h % 2) * DH
                    q_h = qk_sb[hp : hp + DH, h // 2, :]
                    k_h = qk_sb[hp : hp + DH, 4 + h // 2, :]

                    expTs = []
                    for jt in range(8):
                        sim = sim_ps.tile([128, N], f32)
                        for ch in range(2):
                            sl = slice(ch * 512, (ch + 1) * 512)
                            mm(
                                sim[:, sl],
                                k_h[:, jt * 128 : (jt + 1) * 128],
                                q_h[:, sl],
                            )
                        expT = expp.tile([128, N], bf16, tag="e")
                        nc.scalar.activation(expT, sim, AF.Exp, scale=SCALE)
                        expTs.append(expT)

                    oa = oa_ps.tile([DH + 1, N], f32)
                    for jt in range(8):
                        for ch in range(2):
                            sl = slice(ch * 512, (ch + 1) * 512)
                            mm(
                                oa[:, sl],
                                vT_sb[:, jt, h, :],
                                expTs[jt][:, sl],
                                start=(jt == 0),
                                stop=(jt == 7),
                            )

                    nc.vector.tensor_copy(rden, oa[DH : DH + 1, :])
                    nc.vector.reciprocal_approx_fast(out=rrec, in_=rden)
                    rd = rdp.tile([1, N], f32r, tag="rd")
                    nc.vector.tensor_copy(rd, rrec)
                    rb = rb_ps.tile([DH, N], f32)
                    for ch in range(2):
                        sl = slice(ch * 512, (ch + 1) * 512)
                        mm(rb[:, sl], ones_row, rd[:, sl])
                    # DVE reads at most one PSUM operand: stage numerator in SBUF
                    oan = oan_p.tile([DH, N], f32)
                    nc.vector.tensor_copy(oan, oa[0:DH, :])
                    nc.vector.tensor_mul(
                        att_sb[hp : hp + DH, h // 2, :], oan, rb
                    )

            # ============ phase D: out projection + residual ============
            with (
                tc.tile_pool(name="evict", bufs=3) as evict,
                tc.tile_pool(name="z_ps", bufs=3, space="PSUM") as z_ps,
            ):
                for ot in range(4):
                    for ch in range(2):
                        sl = slice(ch * 512, (ch + 1) * 512)
                        ps = z_ps.tile([128, 512], f32)
                        for kt in range(4):
                            mm(
                                ps,
                                woT[:, kt, ot * 128 : (ot + 1) * 128],
                                att_sb[:, kt, sl],
                                start=(kt == 0),
                                stop=(kt == 3),
                            )
                        yt = evict.tile([128, 512], f32)
                        nc.vector.tensor_add(yt, ps, x_sb[:, ot, sl])
                        nc.sync.dma_start(
                            out=y_ap[ot * 128 : (ot + 1) * 128, sl], in_=yt
                        )

    nc.compile()
    return nc


def _get_nc():
    if "nc" not in _CACHE:
        _CACHE["nc"] = _build()
    return _CACHE["nc"]


def kernel(x, gamma, w_qkv, w_out):
    from concourse.bass_utils import run_bass_kernel_spmd

    x = np.ascontiguousarray(x, dtype=np.float32)
    gamma_f = np.ascontiguousarray(gamma, dtype=np.float32).reshape(C)
    wq = np.ascontiguousarray(w_qkv, dtype=np.float32)
    wo = np.ascontiguousarray(w_out, dtype=np.float32)

    nc = _get_nc()
    in_maps = [
        {
            "x": x[b].reshape(C, N).copy(),
            "gamma": gamma_f,
            "w_qkv": wq,
            "w_out": wo,
        }
        for b in range(NCORES)
    ]
    res = run_bass_kernel_spmd(nc, in_maps, core_ids=list(range(NCORES)))
    out = np.stack(
        [res.results[b]["y"].reshape(C, 32, 32) for b in range(NCORES)], axis=0
    )
    return out.astype(np.float32)
